# revision 5
# baseline (speedup 1.0000x reference)
"""ClusteredLogSoftmax Trainium2 kernel.

Math (per reference):
  ex   = exp(logits)                                  [B, V]
  sums[c, b] = sum_{v: cluster[v]=c} ex[b, v]
  out[b, v]  = logits[b, v] - log(sums[cluster[v], b])
  out[b, v]  = log_sigmoid(logits[b, v])   where cluster[v] == 0

Strategy:
  - Data-parallel over batch: 1024 rows -> 8 cores x 128 rows (= 128 SBUF
    partitions, the natural layout).
  - cluster_index is host-visible at kernel-build time, so the whole segment
    structure is compile-time constant.  On the host we reorder the vocab
    columns: cluster 0 first, remaining clusters sorted by size so that
    equal-size clusters are adjacent; every cluster is padded to an even
    number of columns with -1e30 (exp -> 0, so sums are unchanged).
  - On device, per ~2K-column chunk (chunk boundaries aligned to cluster
    boundaries): ACT computes exp; DVE reduces each run of equal-length
    clusters with a single 3D-access-pattern tensor_reduce; ACT takes Ln of
    the per-cluster sums (Exp+Ln share one ACT table set); DVE subtracts the
    per-cluster log-denominator with a stride-0 broadcast tensor_sub.
    Cluster-0 columns are overridden with -softplus(-x).
  - Output columns are scattered back to the original vocab order on host.
"""

import os
import numpy as np

import concourse.bass as bass
import concourse.bacc as bacc
import concourse.tile as tile
import concourse.mybir as mybir
import concourse.bass_utils as bass_utils
from contextlib import ExitStack

F32 = mybir.dt.float32
PAD_VAL = -1e30
N_CORES = 8
CHUNK_COLS = 2048
BIG_COLS = 8192


class _Chunk:
    __slots__ = ("start", "width", "c0_width", "n_red", "runs")

    def __init__(self):
        self.start = 0        # global padded-column start
        self.width = 0
        self.c0_width = 0     # width of the cluster-0 segment (only chunk 0)
        self.n_red = 0        # number of reduced clusters in this chunk
        self.runs = []        # (j0, k, L, s_local): k clusters of length L


class _Plan:
    __slots__ = ("Vp", "src_idx", "chunks", "bigs", "max_nred", "max_chunk_w")


def _build_plan(cluster_index, n_clusters, chunk_cols=CHUNK_COLS, big_cols=BIG_COLS):
    ci = np.asarray(cluster_index).astype(np.int64).ravel()
    C = int(n_clusters)
    counts = np.bincount(ci, minlength=C)

    order = [c for c in range(1, C) if counts[c] > 0]
    order.sort(key=lambda c: (int(counts[c]), c))
    has_c0 = counts[0] > 0
    if has_c0:
        order = [0] + order

    pos = np.full(C, C + 1, np.int64)
    for i, c in enumerate(order):
        pos[c] = i
    sorted_v = np.argsort(pos[ci], kind="stable")

    src_parts = []
    clusters = []  # (cluster_id, Lpad)
    vpos = 0
    for c in order:
        cnt = int(counts[c])
        src_parts.append(sorted_v[vpos : vpos + cnt])
        vpos += cnt
        Lp = cnt + (cnt & 1)
        if cnt & 1:
            # -1: pad with PAD_VAL (exp -> 0); -2: pad for cluster 0 with
            # +1e30 (exp(-x) -> 0 in the log-sigmoid branch)
            src_parts.append(np.array([-1 if c != 0 else -2], np.int64))
        clusters.append((c, Lp))
    src_idx = (
        np.concatenate(src_parts) if src_parts else np.zeros((0,), np.int64)
    )
    Vp = int(src_idx.shape[0])

    # --- chunks: packs of whole clusters, <= chunk_cols wide when possible ---
    chunks = []
    cur = None
    col = 0
    for idx, (c, Lp) in enumerate(clusters):
        if cur is None or (cur.width + Lp > chunk_cols and cur.width > 0):
            cur = _Chunk()
            cur.start = col
            chunks.append(cur)
        if c == 0:
            cur.c0_width = Lp
        else:
            s_local = cur.width
            if cur.runs and cur.runs[-1][2] == Lp and (
                cur.runs[-1][3] + cur.runs[-1][1] * cur.runs[-1][2] == s_local
            ):
                j0, k, L, s0 = cur.runs[-1]
                cur.runs[-1] = (j0, k + 1, L, s0)
            else:
                cur.runs.append((cur.n_red, 1, Lp, s_local))
            cur.n_red += 1
        cur.width += Lp
        col += Lp
    assert col == Vp

    # --- big chunks: DMA granularity ---
    bigs = []  # (start, width, [chunk indices])
    cur_b = None
    for i, ch in enumerate(chunks):
        if cur_b is None or (cur_b[1] + ch.width > big_cols and cur_b[1] > 0):
            cur_b = [ch.start, 0, []]
            bigs.append(cur_b)
        cur_b[1] += ch.width
        cur_b[2].append(i)

    plan = _Plan()
    plan.Vp = Vp
    plan.src_idx = src_idx
    plan.chunks = chunks
    plan.bigs = [(b[0], b[1], b[2]) for b in bigs]
    plan.max_nred = max((c.n_red for c in chunks), default=1)
    plan.max_chunk_w = max((c.width for c in chunks), default=1)
    return plan


def _build_kernel(plan, repeat=1):
    nc = bacc.Bacc("TRN2", target_bir_lowering=False, debug=False)
    Vp = plan.Vp
    x = nc.dram_tensor("x", [128, Vp], F32, kind="ExternalInput").ap()
    y = nc.dram_tensor("y", [128, Vp], F32, kind="ExternalOutput").ap()

    AF = mybir.ActivationFunctionType
    AX = mybir.AxisListType
    OP = mybir.AluOpType

    with ExitStack() as ctx:
        tc = ctx.enter_context(tile.TileContext(nc))
        pin = ctx.enter_context(tc.tile_pool(name="pin", bufs=2))
        pout = ctx.enter_context(tc.tile_pool(name="pout", bufs=2))
        pex = ctx.enter_context(tc.tile_pool(name="pex", bufs=3))
        psmall = ctx.enter_context(tc.tile_pool(name="psmall", bufs=4))

        for _ in range(repeat):
            for (bstart, bwidth, chunk_ids) in plan.bigs:
                bin_t = pin.tile([128, bwidth], F32, tag="bin")
                nc.sync.dma_start(bin_t[:, :], x[:, bstart : bstart + bwidth])
                bout_t = pout.tile([128, bwidth], F32, tag="bout")

                for ci_ in chunk_ids:
                    ch = plan.chunks[ci_]
                    off = ch.start - bstart
                    # main exp skips the cluster-0 segment (handled below)
                    e0 = ch.c0_width
                    w = ch.width
                    ex = pex.tile([128, plan.max_chunk_w], F32, tag="ex")
                    if w > e0:
                        nc.scalar.activation(
                            ex[:, e0:w], bin_t[:, off + e0 : off + w], AF.Exp
                        )
                    if ch.n_red:
                        n = ch.n_red
                        sums = psmall.tile([128, plan.max_nred], F32, tag="sums")
                        for (j0, k, L, s_local) in ch.runs:
                            src = ex[:, s_local : s_local + k * L].rearrange(
                                "p (k l) -> p k l", k=k
                            )
                            nc.vector.tensor_reduce(
                                sums[:, j0 : j0 + k], src, axis=AX.X, op=OP.add
                            )
                        lsum = psmall.tile([128, plan.max_nred], F32, tag="lsum")
                        nc.scalar.activation(lsum[:, :n], sums[:, :n], AF.Ln)
                        for (j0, k, L, s_local) in ch.runs:
                            g0 = off + s_local
                            i3 = bin_t[:, g0 : g0 + k * L].rearrange(
                                "p (k l) -> p k l", k=k
                            )
                            o3 = bout_t[:, g0 : g0 + k * L].rearrange(
                                "p (k l) -> p k l", k=k
                            )
                            b3 = (
                                lsum[:, j0 : j0 + k]
                                .unsqueeze(2)
                                .broadcast_to([128, k, L])
                            )
                            nc.vector.tensor_sub(o3, i3, b3)
                    if ch.c0_width:
                        # log_sigmoid(x) = -ln(1 + exp(-x))
                        w0 = ch.c0_width
                        sp = psmall.tile([128, w0], F32, tag="sp")
                        sp2 = psmall.tile([128, w0], F32, tag="sp2")
                        nc.scalar.activation(
                            sp[:, :], bin_t[:, off : off + w0], AF.Exp,
                            scale=-1.0,
                        )
                        nc.vector.tensor_scalar_add(sp2[:, :], sp[:, :], 1.0)
                        nc.scalar.activation(sp[:, :], sp2[:, :], AF.Ln)
                        nc.vector.tensor_scalar_mul(
                            bout_t[:, off : off + w0], sp[:, :], -1.0
                        )

                nc.sync.dma_start(y[:, bstart : bstart + bwidth], bout_t[:, :])

    nc.compile()
    return nc


def _prepare_shards(logits, plan):
    B, V = logits.shape
    assert B % N_CORES == 0
    rows = B // N_CORES
    valid = plan.src_idx >= 0
    sorted_full = np.empty((B, plan.Vp), np.float32)
    sorted_full[:, valid] = logits[:, plan.src_idx[valid]]
    sorted_full[:, plan.src_idx == -1] = PAD_VAL
    sorted_full[:, plan.src_idx == -2] = -PAD_VAL
    return [
        np.ascontiguousarray(sorted_full[i * rows : (i + 1) * rows])
        for i in range(N_CORES)
    ]


def _scatter_back(out_sorted, plan, B, V):
    valid = plan.src_idx >= 0
    out = np.empty((B, V), np.float32)
    out[:, plan.src_idx[valid]] = out_sorted[:, valid]
    return out


def kernel(logits, cluster_index, n_clusters, _repeat=1, _return_raw=False):
    logits = np.asarray(logits, dtype=np.float32)
    B, V = logits.shape
    C = int(np.asarray(n_clusters))

    plan = _build_plan(cluster_index, C)
    nc = _build_kernel(plan, repeat=_repeat)
    shards = _prepare_shards(logits, plan)
    res = bass_utils.run_bass_kernel_spmd(
        nc, [{"x": s} for s in shards], core_ids=list(range(N_CORES))
    )
    out_sorted = np.concatenate([res.results[i]["y"] for i in range(N_CORES)], axis=0)
    if _return_raw:
        return out_sorted, plan
    return _scatter_back(out_sorted, plan, B, V)


# revision 16
# speedup vs baseline: 1056.3359x; 1056.3359x over previous
"""ClusteredLogSoftmax Trainium2 kernel.

Math (per reference):
  ex   = exp(logits)                                  [B, V]
  sums[c, b] = sum_{v: cluster[v]=c} ex[b, v]
  out[b, v]  = logits[b, v] - log(sums[cluster[v], b])
  out[b, v]  = log_sigmoid(logits[b, v])   where cluster[v] == 0

Strategy:
  - Data-parallel over batch: 1024 rows -> 8 cores x 128 rows (= 128 SBUF
    partitions, the natural layout).
  - cluster_index is host-visible at kernel-build time, so the whole segment
    structure is compile-time constant.  On the host we reorder the vocab
    columns: cluster 0 first, remaining clusters sorted by size so that
    equal-size clusters are adjacent; every cluster is padded to an even
    number of columns with -1e30 (exp -> 0, so sums are unchanged).
  - On device, per ~2K-column chunk (chunk boundaries aligned to cluster
    boundaries): ACT computes exp; DVE reduces each run of equal-length
    clusters with a single 3D-access-pattern tensor_reduce; ACT takes Ln of
    the per-cluster sums (Exp+Ln share one ACT table set); DVE subtracts the
    per-cluster log-denominator with a stride-0 broadcast tensor_sub.
    Cluster-0 columns are overridden with -softplus(-x).
  - Output columns are scattered back to the original vocab order on host.
"""

import os
import numpy as np

import concourse.bass as bass
import concourse.bacc as bacc
import concourse.tile as tile
import concourse.mybir as mybir
import concourse.bass_utils as bass_utils
from contextlib import ExitStack

F32 = mybir.dt.float32
PAD_VAL = -1e30
N_CORES = 8
CHUNK_COLS = 2048
BIG_COLS = 4096


class _Bacc(bacc.Bacc):
    """Bacc whose activation-table-load pass is steered to the combined
    natural_log_exp_and_others set.

    The stock greedy pass resolves Exp via `exp_and_others` and Ln via
    `natural_log`, reloading ACT tables on every Exp<->Ln alternation
    (~2.7us each, ~once per chunk).  Both functions live together in
    `natural_log_exp_and_others`; blanking the exp-only/ln-only sets (list
    positions preserved, so act_func_set_id indices stay valid) makes the
    pass hoist a single load.
    """

    _ACT_SET_BLANKLIST = {"exp_and_others", "natural_log", "exp_and_friends"}

    def insert_act_table_loads(self):
        from concourse.hw_specs import get_activation_tables

        has_activation = any(
            isinstance(i, mybir.InstActivation)
            for b in self.main_func.blocks
            for i in b.instructions
        )
        if not has_activation:
            return
        tables = [
            (name, set() if name in self._ACT_SET_BLANKLIST else funcs)
            for name, funcs in get_activation_tables(self.m.arch).items()
        ]
        import bass_rust as _bass_rust

        _bass_rust.insert_act_table_loads(self, tables)


class _Chunk:
    __slots__ = ("start", "width", "c0_width", "n_red", "runs")

    def __init__(self):
        self.start = 0        # global padded-column start
        self.width = 0
        self.c0_width = 0     # width of the cluster-0 segment (only chunk 0)
        self.n_red = 0        # number of reduced clusters in this chunk
        self.runs = []        # (j0, k, L, s_local): k clusters of length L


class _Plan:
    __slots__ = ("Vp", "src_idx", "chunks", "bigs", "max_nred", "max_chunk_w")


def _build_plan(cluster_index, n_clusters, chunk_cols=CHUNK_COLS, big_cols=BIG_COLS):
    ci = np.asarray(cluster_index).astype(np.int64).ravel()
    C = int(n_clusters)
    counts = np.bincount(ci, minlength=C)

    order = [c for c in range(1, C) if counts[c] > 0]
    order.sort(key=lambda c: (int(counts[c]), c))
    has_c0 = counts[0] > 0
    if has_c0:
        order = [0] + order

    pos = np.full(C, C + 1, np.int64)
    for i, c in enumerate(order):
        pos[c] = i
    sorted_v = np.argsort(pos[ci], kind="stable")

    src_parts = []
    clusters = []  # (cluster_id, Lpad)
    vpos = 0
    for c in order:
        cnt = int(counts[c])
        src_parts.append(sorted_v[vpos : vpos + cnt])
        vpos += cnt
        Lp = cnt + (cnt & 1)
        if cnt & 1:
            # -1: pad with PAD_VAL (exp -> 0); -2: pad for cluster 0 with
            # +1e30 (exp(-x) -> 0 in the log-sigmoid branch)
            src_parts.append(np.array([-1 if c != 0 else -2], np.int64))
        clusters.append((c, Lp))
    src_idx = (
        np.concatenate(src_parts) if src_parts else np.zeros((0,), np.int64)
    )
    Vp = int(src_idx.shape[0])

    # --- chunks: packs of whole clusters, <= chunk_cols wide when possible ---
    chunks = []
    cur = None
    col = 0
    for idx, (c, Lp) in enumerate(clusters):
        if cur is None or (cur.width + Lp > chunk_cols and cur.width > 0):
            cur = _Chunk()
            cur.start = col
            chunks.append(cur)
        if c == 0:
            cur.c0_width = Lp
        else:
            s_local = cur.width
            if cur.runs and cur.runs[-1][2] == Lp and (
                cur.runs[-1][3] + cur.runs[-1][1] * cur.runs[-1][2] == s_local
            ):
                j0, k, L, s0 = cur.runs[-1]
                cur.runs[-1] = (j0, k + 1, L, s0)
            else:
                cur.runs.append((cur.n_red, 1, Lp, s_local))
            cur.n_red += 1
        cur.width += Lp
        col += Lp
    assert col == Vp

    # --- big chunks: DMA granularity ---
    bigs = []  # (start, width, [chunk indices])
    cur_b = None
    for i, ch in enumerate(chunks):
        if cur_b is None or (cur_b[1] + ch.width > big_cols and cur_b[1] > 0):
            cur_b = [ch.start, 0, []]
            bigs.append(cur_b)
        cur_b[1] += ch.width
        cur_b[2].append(i)

    plan = _Plan()
    plan.Vp = Vp
    plan.src_idx = src_idx
    plan.chunks = chunks
    plan.bigs = [(b[0], b[1], b[2]) for b in bigs]
    plan.max_nred = max((c.n_red for c in chunks), default=1)
    plan.max_chunk_w = max((c.width for c in chunks), default=1)
    return plan


def _build_kernel(plan, repeat=1, in_bufs=5, out_bufs=5, loop_n=0):
    nc = _Bacc("TRN2", target_bir_lowering=False, debug=False)
    Vp = plan.Vp
    x = nc.dram_tensor("x", [128, Vp], F32, kind="ExternalInput").ap()
    y = nc.dram_tensor("y", [128, Vp], F32, kind="ExternalOutput").ap()

    AF = mybir.ActivationFunctionType
    AX = mybir.AxisListType
    OP = mybir.AluOpType

    with ExitStack() as ctx:
        tc = ctx.enter_context(tile.TileContext(nc))
        pin = ctx.enter_context(tc.tile_pool(name="pin", bufs=in_bufs))
        pout = ctx.enter_context(tc.tile_pool(name="pout", bufs=out_bufs))
        psmall = ctx.enter_context(tc.tile_pool(name="psmall", bufs=4))

        def one_pass():
            for (bstart, bwidth, chunk_ids) in plan.bigs:
                bin_t = pin.tile([128, bwidth], F32, tag="bin")
                nc.sync.dma_start(bin_t[:, :], x[:, bstart : bstart + bwidth])
                bout_t = pout.tile([128, bwidth], F32, tag="bout")

                for ci_ in chunk_ids:
                    ch = plan.chunks[ci_]
                    off = ch.start - bstart
                    # main exp skips the cluster-0 segment (handled below).
                    # exp is written into the OUT tile; the reduces read it
                    # there, after which the subtract overwrites it in place
                    # (Tile's WAR tracking orders reduce -> sub).
                    e0 = ch.c0_width
                    w = ch.width
                    if w > e0:
                        nc.scalar.activation(
                            bout_t[:, off + e0 : off + w],
                            bin_t[:, off + e0 : off + w],
                            AF.Exp,
                        )
                    if ch.n_red:
                        n = ch.n_red
                        sums = psmall.tile([128, plan.max_nred], F32, tag="sums")
                        for (j0, k, L, s_local) in ch.runs:
                            src = bout_t[
                                :, off + s_local : off + s_local + k * L
                            ].rearrange("p (k l) -> p k l", k=k)
                            nc.vector.tensor_reduce(
                                sums[:, j0 : j0 + k], src, axis=AX.X, op=OP.add
                            )
                        lsum = psmall.tile([128, plan.max_nred], F32, tag="lsum")
                        nc.scalar.activation(lsum[:, :n], sums[:, :n], AF.Ln)
                        for (j0, k, L, s_local) in ch.runs:
                            g0 = off + s_local
                            i3 = bin_t[:, g0 : g0 + k * L].rearrange(
                                "p (k l) -> p k l", k=k
                            )
                            o3 = bout_t[:, g0 : g0 + k * L].rearrange(
                                "p (k l) -> p k l", k=k
                            )
                            b3 = (
                                lsum[:, j0 : j0 + k]
                                .unsqueeze(2)
                                .broadcast_to([128, k, L])
                            )
                            nc.vector.tensor_sub(o3, i3, b3)
                    if ch.c0_width:
                        # log_sigmoid(x) = -ln(1 + exp(-x))
                        w0 = ch.c0_width
                        sp = psmall.tile([128, w0], F32, tag="sp")
                        sp2 = psmall.tile([128, w0], F32, tag="sp2")
                        nc.scalar.activation(
                            sp[:, :], bin_t[:, off : off + w0], AF.Exp,
                            scale=-1.0,
                        )
                        nc.vector.tensor_scalar_add(sp2[:, :], sp[:, :], 1.0)
                        nc.scalar.activation(sp[:, :], sp2[:, :], AF.Ln)
                        nc.vector.tensor_scalar_mul(
                            bout_t[:, off : off + w0], sp[:, :], -1.0
                        )

                # out-DMAs ride the ACT HWDGE ring so they run concurrently
                # with the in-DMAs on the SP ring
                nc.scalar.dma_start(y[:, bstart : bstart + bwidth], bout_t[:, :])

        if loop_n > 1:
            with tc.For_i(0, loop_n, 1):
                one_pass()
        else:
            for _ in range(repeat):
                one_pass()

    nc.compile()
    return nc


def _prepare_shards(logits, plan):
    B, V = logits.shape
    assert B % N_CORES == 0
    rows = B // N_CORES
    valid = plan.src_idx >= 0
    sorted_full = np.empty((B, plan.Vp), np.float32)
    sorted_full[:, valid] = logits[:, plan.src_idx[valid]]
    sorted_full[:, plan.src_idx == -1] = PAD_VAL
    sorted_full[:, plan.src_idx == -2] = -PAD_VAL
    return [
        np.ascontiguousarray(sorted_full[i * rows : (i + 1) * rows])
        for i in range(N_CORES)
    ]


def _scatter_back(out_sorted, plan, B, V):
    valid = plan.src_idx >= 0
    out = np.empty((B, V), np.float32)
    out[:, plan.src_idx[valid]] = out_sorted[:, valid]
    return out


def kernel(logits, cluster_index, n_clusters, _repeat=1, _return_raw=False):
    logits = np.asarray(logits, dtype=np.float32)
    B, V = logits.shape
    C = int(np.asarray(n_clusters))

    plan = _build_plan(cluster_index, C)
    nc = _build_kernel(plan, repeat=_repeat)
    shards = _prepare_shards(logits, plan)
    res = bass_utils.run_bass_kernel_spmd(
        nc, [{"x": s} for s in shards], core_ids=list(range(N_CORES))
    )
    out_sorted = np.concatenate([res.results[i]["y"] for i in range(N_CORES)], axis=0)
    if _return_raw:
        return out_sorted, plan
    return _scatter_back(out_sorted, plan, B, V)
